# revision 1
# baseline (speedup 1.0000x reference)
"""Contrastive-loss Trainium2 kernel: symmetry-halved fp8 DoubleRow GEMM + AllGather.

zn is scaled by 16 before the fp8 cast (values ~0.5 fit e4m3 comfortably);
the GEMM result is 256*cos, compensated in the exp scale and pos scale.

cos_sim is symmetric, so only block-distances d ∈ {0..4} are computed per
core (columns local [0:5120) in the rotated frame); the exp-sums for
distances 5,6,7 of each row are the COLUMN sums of the d ∈ {3,2,1} blocks
computed by cores c+5, c+6, c+7. Each core:
  - computes its 1024 x 5120 block of exp(S/T) with diag masked,
  - row-sums it (ACT fused accum),
  - column-sums the d in {1,2,3} sub-blocks (elementwise accumulate over
    the 8 row-tiles on DVE, then a partition-reduce via a ones-matmul),
  - AllGathers packet = [rowsums(1024), cs_d1(1024), cs_d2(1024),
    cs_d3(1024)] (fp32, 16KB/rank),
  - reassembles the GLOBAL per-row totals (the gathered frame is
    rank-indexed, so placement is static), takes ln, and reduces
    Sum_r ln(total_r) over all 8192 rows (identical on every core).
Output per core: [128,1] partial = (Sum ln)/8 - Sum_own(pos)/T partials.
Host sums 8x128 values / 8192.
"""

import os
from contextlib import ExitStack

import numpy as np

N = 8192
D = 1024
N_CORES = 8
ROWS_PER_CORE = N // N_CORES  # 1024
P = 128
TEMPERATURE = 0.07
INV_T = 1.0 / TEMPERATURE
MASK_VAL = -65504.0
SCALE = 16.0  # pre-fp8 scale; psum holds SCALE^2 * cos

NBLK = 5  # block distances 0..4 computed locally
COLS = NBLK * ROWS_PER_CORE  # 5120 local columns
N_ROW_TILES = COLS // P  # 40 row tiles to normalize (rows [0:5120))
MB = ROWS_PER_CORE // P  # 8
KT = D // P  # 8
COLG = 512
NB = COLS // COLG  # 10 column tiles
CS_NB = range(2, 8)  # col tiles covering d in {1,2,3} (cols 1024:4096)
PKT = 4 * ROWS_PER_CORE  # packet floats: rowsum + 3 colsum blocks

_CACHE = {}


def _build_nc(repeat=1):
    import concourse.mybir as mybir
    import concourse.tile as tile
    from concourse import bacc
    from concourse.masks import make_identity

    f32 = mybir.dt.float32
    bf16 = mybir.dt.bfloat16
    fp8 = mybir.dt.float8e4
    AF = mybir.ActivationFunctionType
    ALU = mybir.AluOpType

    nc = bacc.Bacc("TRN2")
    z_in = nc.dram_tensor("z", [N, D], f32, kind="ExternalInput")
    out_dram = nc.dram_tensor("out", [P, 1], f32, kind="ExternalOutput")
    pkt_dram = nc.dram_tensor("pkt", [PKT], f32)
    gathered = nc.dram_tensor("gathered", [N_CORES, PKT], f32, addr_space="Shared")

    ctx = ExitStack()
    with ctx:
        tc = ctx.enter_context(tile.TileContext(nc))
        consts = ctx.enter_context(tc.tile_pool(name="consts", bufs=1))
        znt_pool = ctx.enter_context(tc.tile_pool(name="znt", bufs=1))
        work = ctx.enter_context(tc.tile_pool(name="work", bufs=3))
        zin = ctx.enter_context(tc.tile_pool(name="zin", bufs=8))
        small = ctx.enter_context(tc.tile_pool(name="small", bufs=4))
        accp = ctx.enter_context(tc.tile_pool(name="accp", bufs=1))
        colp = ctx.enter_context(tc.tile_pool(name="colp", bufs=1))
        psum_t = ctx.enter_context(tc.tile_pool(name="psum_t", bufs=2, space="PSUM"))
        psum_mm = ctx.enter_context(tc.tile_pool(name="psum_mm", bufs=4, space="PSUM"))
        psum_cs = ctx.enter_context(tc.tile_pool(name="psum_cs", bufs=2, space="PSUM"))

        ident_f32 = consts.tile([P, P], f32, tag="ident_f32")
        make_identity(nc, ident_f32)
        ident_bf16 = consts.tile([P, P], bf16, tag="ident_bf16")
        make_identity(nc, ident_bf16)
        ident_fp8 = consts.tile([P, P], fp8, tag="ident_fp8")
        make_identity(nc, ident_fp8)
        negtile = consts.tile([P, P], f32, tag="negtile")
        nc.vector.memset(negtile, MASK_VAL * SCALE * SCALE)
        ident_u8 = consts.tile([P, P], mybir.dt.uint8, tag="ident_u8")
        nc.vector.tensor_copy(ident_u8, ident_f32)
        ones_col = consts.tile([P, 1], bf16, tag="ones_col")
        nc.vector.memset(ones_col, 1.0)

        znt = [
            znt_pool.tile([P, KT, COLG], fp8, tag=f"znt{g}", name=f"znt{g}")
            for g in range(NB)
        ]

        accs = accp.tile([P, MB, NB], f32, tag="accs")
        posq = accp.tile([P, MB], f32, tag="posq")
        # colT[j] accumulates sum over the 8 row-tiles of exp'd tile nb=2+j
        colT = [
            colp.tile([P, COLG], f32, tag=f"colT{j}", name=f"colT{j}")
            for j in range(len(CS_NB))
        ]
        cs = colp.tile([P, 24], f32, tag="cs")  # colsums, col c = local col chunk

        for _rep in range(repeat):
            # ---- phase 1: normalize + transpose (rows [0:5120) only) ----
            for t in range(N_ROW_TILES):
                zt = zin.tile([P, 2, D // 2], f32, tag="zt")
                nc.sync.dma_start(
                    out=zt,
                    in_=z_in[t * P : (t + 1) * P, :].rearrange(
                        "p (a b) -> p a b", a=2
                    ),
                )
                stats = small.tile([P, 2, 6], f32, tag="stats")
                nc.vector.bn_stats(out=stats[:, 0, :], in_=zt[:, 0, :])
                nc.vector.bn_stats(out=stats[:, 1, :], in_=zt[:, 1, :])
                mv = small.tile([P, 2], f32, tag="mv")
                nc.vector.bn_aggr(out=mv, in_=stats)
                m2 = small.tile([P, 1], f32, tag="m2")
                nc.vector.tensor_mul(m2, mv[:, 0:1], mv[:, 0:1])
                s2 = small.tile([P, 1], f32, tag="s2")
                nc.vector.tensor_add(s2, m2, mv[:, 1:2])
                nrm = small.tile([P, 1], f32, tag="nrm")
                nc.scalar.activation(nrm, s2, AF.Sqrt, scale=float(D) / (SCALE * SCALE))
                rinv = small.tile([P, 1], f32, tag="rinv")
                nc.vector.reciprocal(rinv, nrm)

                zn_row = work.tile([P, D], bf16, tag="zn_row")
                nc.vector.tensor_scalar_mul(
                    zn_row.rearrange("p (a b) -> p a b", a=2), zt, rinv
                )

                ptr = psum_t.tile([P, KT * P], bf16, tag="ptr")
                for kk in range(KT):
                    nc.tensor.transpose(
                        ptr[:, kk * P : (kk + 1) * P],
                        zn_row[:, kk * P : (kk + 1) * P],
                        ident_bf16,
                    )
                g, col = t // 4, (t % 4) * P
                dst = znt[g][:, :, col : col + P]
                src = ptr.rearrange("p (k c) -> p k c", k=KT)
                if t % 2 == 0:
                    nc.scalar.copy(dst, src)
                else:
                    nc.vector.tensor_copy(dst, src)

            # zero colsum accumulators
            for j in range(len(CS_NB)):
                nc.vector.memset(colT[j], 0.0)

            # ---- phase 2: GEMM + exp row-sums + colsum accumulation ----
            for nb in range(NB):
                for mb in range(MB):
                    ps = psum_mm.tile([P, COLG], f32, tag="ps")
                    lg, lcol = mb // 4, (mb % 4) * P
                    for kk in range(0, KT, 2):
                        nc.tensor.matmul(
                            ps,
                            lhsT=znt[lg][:, kk : kk + 2, lcol : lcol + P],
                            rhs=znt[nb][:, kk : kk + 2, :],
                            perf_mode=mybir.MatmulPerfMode.DoubleRow,
                            start=(kk == 0),
                            stop=(kk == KT - 2),
                        )
                    if nb == mb // 4:
                        off = (mb % 4) * P
                        nc.vector.copy_predicated(
                            out=ps[:, off : off + P], mask=ident_u8, data=negtile
                        )
                    if nb == 8 + mb // 4:
                        off = (mb % 4) * P
                        pos_scr = work.tile([P, P], f32, tag="pos_scr")
                        nc.vector.tensor_mul(pos_scr, ps[:, off : off + P], ident_f32)
                        nc.vector.tensor_reduce(
                            posq[:, mb : mb + 1],
                            pos_scr,
                            axis=mybir.AxisListType.X,
                            op=ALU.add,
                        )
                    ex = work.tile([P, COLG], bf16, tag="ex")
                    nc.scalar.activation(
                        ex, ps, AF.Exp, scale=INV_T / (SCALE * SCALE),
                        accum_out=accs[:, mb, nb : nb + 1],
                    )
                    if nb in CS_NB:
                        j = nb - 2
                        nc.vector.tensor_add(colT[j], colT[j], ex)

            # ---- colsum partition-reduce via ones-matmul ----
            for j in range(len(CS_NB)):
                ctb = work.tile([P, COLG], bf16, tag="ctb")
                nc.vector.tensor_copy(ctb, colT[j])
                for q in range(COLG // P):
                    cps = psum_cs.tile([P, 1], f32, tag="cps")
                    nc.tensor.matmul(
                        cps,
                        lhsT=ctb[:, q * P : (q + 1) * P],
                        rhs=ones_col,
                        start=True,
                        stop=True,
                    )
                    nc.scalar.copy(cs[:, 4 * j + q : 4 * j + q + 1], cps)

            # ---- pack + AllGather ----
            rowsum = accp.tile([P, MB], f32, tag="rowsum")
            nc.vector.tensor_reduce(
                rowsum, accs, axis=mybir.AxisListType.X, op=ALU.add
            )
            d1 = nc.sync.dma_start(
                out=pkt_dram[0:ROWS_PER_CORE].rearrange("(m p) -> p m", p=P),
                in_=rowsum,
            )
            d2 = nc.sync.dma_start(
                out=pkt_dram[ROWS_PER_CORE:PKT].rearrange("(c p) -> p c", p=P),
                in_=cs,
            )
            cc = nc.gpsimd.collective_compute(
                "AllGather",
                mybir.AluOpType.bypass,
                ins=[pkt_dram.ap()],
                outs=[gathered.ap()],
                replica_groups=[list(range(N_CORES))],
            )
            from concourse.bass import _add_dep_helper

            _add_dep_helper(cc.ins, d1.ins, reason="cc after pkt rowsum")
            _add_dep_helper(cc.ins, d2.ins, reason="cc after pkt cs")

            # ---- reassemble global totals; ln; global reduce ----
            # tot[p, b, m] = total exp-sum for global row 1024 b + 128 m + p
            Rt = accp.tile([P, N_CORES, MB], f32, tag="Rt")
            tot = accp.tile([P, N_CORES, MB], f32, tag="tot")
            Cd = {
                d: accp.tile([P, N_CORES, MB], f32, tag=f"Cd{d}", name=f"Cd{d}")
                for d in (1, 2, 3)
            }
            for b in range(N_CORES):
                dr = nc.sync.dma_start(
                    out=Rt[:, b, :],
                    in_=gathered[b, 0:ROWS_PER_CORE].rearrange("(m p) -> p m", p=P),
                )
                _add_dep_helper(dr.ins, cc.ins, reason="read gathered after cc")
                for d in (1, 2, 3):
                    s = (b - d) % N_CORES
                    dc = nc.sync.dma_start(
                        out=Cd[d][:, b, :],
                        in_=gathered[
                            s, d * ROWS_PER_CORE : (d + 1) * ROWS_PER_CORE
                        ].rearrange("(m p) -> p m", p=P),
                    )
                    _add_dep_helper(dc.ins, cc.ins, reason="read gathered after cc")
            nc.vector.tensor_copy(tot, Rt)
            for d in (1, 2, 3):
                nc.vector.tensor_add(tot, tot, Cd[d])

            lnt = accp.tile([P, N_CORES, MB], f32, tag="lnt")
            nc.scalar.activation(lnt, tot, AF.Ln)
            gsum = accp.tile([P, 1], f32, tag="gsum")
            nc.vector.tensor_reduce(
                gsum, lnt, axis=mybir.AxisListType.XY, op=ALU.add
            )
            poss = accp.tile([P, MB], f32, tag="poss")
            nc.vector.tensor_scalar_mul(poss, posq, -INV_T / (SCALE * SCALE))
            psum_part = accp.tile([P, 1], f32, tag="psum_part")
            nc.vector.tensor_reduce(
                psum_part, poss, axis=mybir.AxisListType.X, op=ALU.add
            )
            part = accp.tile([P, 1], f32, tag="part")
            nc.vector.tensor_scalar_mul(part, gsum, 1.0 / N_CORES)
            nc.vector.tensor_add(part, part, psum_part)
            nc.sync.dma_start(out=out_dram[:, :], in_=part)

    nc.finalize()
    return nc


def _get_nc():
    if "nc" not in _CACHE:
        _CACHE["nc"] = _build_nc()
    return _CACHE["nc"]


def _run(z, trace=False):
    from concourse.bass_utils import run_bass_kernel_spmd

    z = np.ascontiguousarray(np.asarray(z, dtype=np.float32))
    assert z.shape == (N, D), z.shape
    nc = _get_nc()
    in_maps = [
        {"z": np.ascontiguousarray(np.roll(z, -ROWS_PER_CORE * c, axis=0))}
        for c in range(N_CORES)
    ]
    res = run_bass_kernel_spmd(
        nc, in_maps, core_ids=list(range(N_CORES)), trace=False
    )
    total = np.float64(0.0)
    for r in res.results:
        total += r["out"].astype(np.float64).sum()
    loss = np.float32(total / N)
    return loss, res


def kernel(z):
    loss, _ = _run(z, trace=False)
    return np.array(loss, dtype=np.float32)



# revision 2
# speedup vs baseline: 18.3135x; 18.3135x over previous
"""Contrastive-loss TRN2 kernel v3: staircase symmetry + gathered fp8 znT.

Math: loss = mean_i[ -cos_sim(z_i, z_{i-N/2})/T + logsumexp_j(cos_sim(z_i,z_j)
with diag masked, /T) ].

Decomposition (per core c of 8, rows R_c = [1024c, 1024c+1024)):
  The 64x64 grid of 128x128 tiles is split by cyclic tile-distance
  delta = (col_tile - row_tile) mod 64. Each row tile computes delta in
  [0, 32] -- a STAIRCASE: row tile t covers local columns
  [128t, 128t+4224) of the 5120-wide span (blocks c..c+4). Entries at
  delta in [33, 63] are recovered from column sums of the transposed
  tiles (computed by the owner of those rows). delta=32 tiles are the
  only double-computed ones (2.4% overhead vs optimal half-matrix).

  Column-sum routing (sender core c, its local col tile v, global
  consumer rows = columns of tile v):
    (a) v in [8, 31]   -> consumer core c+1..c+3, all 8 row tiles: full
        colsums, exported (24 x 128 floats).
    (b) v in [32, 38]  -> consumer (c+4, t=v-32), row tiles l>t only:
        partial colsums, exported (7 x 128).
    (c) v in [1, 7]    -> own rows (tile v), row tiles l<v: folded into
        the local rowsum before export.
  Packet = rowsum(1024) + (a)(3072) + (b)(896) = 4992 f32 per rank.

Data movement: each core loads ONLY its own 1024 rows (4MB f32),
normalizes + transposes + casts to fp8 (x16 scale), AllGathers the 1MB
znT slab, then pulls the 4 neighbour slabs (c+1..c+4) out of the
gathered buffer with per-core indirect DMAs (offset tensor supplies the
rotation; static code stays SPMD-uniform).

Output per core: [128,1] partials; host sums / 8192 (identical global
term on every core for the logsumexp mean, plus the core-local pos part).
"""

import os
from contextlib import ExitStack

import numpy as np

N = 8192
D = 1024
N_CORES = 8
ROWS_PER_CORE = N // N_CORES  # 1024
P = 128
MB = ROWS_PER_CORE // P  # 8 row tiles
KT = D // P  # 8
TEMPERATURE = 0.07
INV_T = 1.0 / TEMPERATURE
MASK_VAL = -65504.0
SCALE = 16.0  # fp8 pre-scale; psum holds SCALE^2 * cos

NTILES = 40          # local col tiles (blocks c..c+4)
WIN = 33             # tile-distances 0..32 computed per row tile
COLS = NTILES * P    # 5120
NSLOT = 31           # exported colsum slots: (a) 24 + (b) 7
PKT = P * NSLOT      # 3968, partition-major [p, slot]

_CACHE = {}


def _pieces(mb):
    """Chunks (abs_start, len<=512) covering window [128mb, 128mb+4224),
    split at 1024 (slab) boundaries."""
    w0, w1 = 128 * mb, 128 * mb + WIN * P
    bounds = [w0] + [1024 * j for j in range(1, 6) if w0 < 1024 * j < w1] + [w1]
    out = []
    for a, b in zip(bounds, bounds[1:]):
        s = a
        while s < b:
            ln = min(512, b - s)
            out.append((s, ln))
            s += ln
    return out


def _build_nc(repeat=1):
    import concourse.mybir as mybir
    import concourse.tile as tile
    from concourse import bacc, bass
    from concourse.bass import _add_dep_helper
    from concourse.masks import make_identity

    f32 = mybir.dt.float32
    bf16 = mybir.dt.bfloat16
    fp8 = mybir.dt.float8e4
    i32 = mybir.dt.int32
    AF = mybir.ActivationFunctionType
    ALU = mybir.AluOpType

    nc = bacc.Bacc("TRN2")
    z_in = nc.dram_tensor("z", [ROWS_PER_CORE, D], f32, kind="ExternalInput")
    goff_in = nc.dram_tensor("goff", [P, 4], i32, kind="ExternalInput")
    poff_in = nc.dram_tensor("poff", [P, 4], i32, kind="ExternalInput")
    out_dram = nc.dram_tensor("out", [P, 1], f32, kind="ExternalOutput")
    pkt_z = nc.dram_tensor("pkt_z", [P, KT, ROWS_PER_CORE], fp8)
    gathered_z = nc.dram_tensor(
        "gathered_z", [N_CORES, P, KT, ROWS_PER_CORE], fp8, addr_space="Shared"
    )
    pkt_dram = nc.dram_tensor("pkt", [PKT], f32)
    gathered = nc.dram_tensor("gathered", [N_CORES, PKT], f32, addr_space="Shared")

    ctx = ExitStack()
    with ctx:
        tc = ctx.enter_context(tile.TileContext(nc))
        consts = ctx.enter_context(tc.tile_pool(name="consts", bufs=1))
        znt_pool = ctx.enter_context(tc.tile_pool(name="znt", bufs=1))
        work = ctx.enter_context(tc.tile_pool(name="work", bufs=3))
        zin = ctx.enter_context(tc.tile_pool(name="zin", bufs=3))
        small = ctx.enter_context(tc.tile_pool(name="small", bufs=4))
        accp = ctx.enter_context(tc.tile_pool(name="accp", bufs=1))
        psum_t = ctx.enter_context(tc.tile_pool(name="psum_t", bufs=2, space="PSUM"))
        psum_mm = ctx.enter_context(tc.tile_pool(name="psum_mm", bufs=4, space="PSUM"))
        psum_cs = ctx.enter_context(tc.tile_pool(name="psum_cs", bufs=2, space="PSUM"))

        ident_f32 = consts.tile([P, P], f32, tag="ident_f32")
        make_identity(nc, ident_f32)
        ident_bf16 = consts.tile([P, P], bf16, tag="ident_bf16")
        make_identity(nc, ident_bf16)
        negtile = consts.tile([P, P], f32, tag="negtile")
        nc.vector.memset(negtile, MASK_VAL * SCALE * SCALE)
        ident_u8 = consts.tile([P, P], mybir.dt.uint8, tag="ident_u8")
        nc.vector.tensor_copy(ident_u8, ident_f32)
        ones_col = consts.tile([P, 1], bf16, tag="ones_col")
        nc.vector.memset(ones_col, 1.0)

        goff_t = consts.tile([P, 4], i32, tag="goff_t")
        nc.sync.dma_start(out=goff_t, in_=goff_in[:, :])
        poff_t = consts.tile([P, 4], i32, tag="poff_t")
        nc.sync.dma_start(out=poff_t, in_=poff_in[:, :])

        # own znT slab + 4 gathered neighbour slabs, each [P, KT, 1024] fp8
        znt0 = znt_pool.tile([P, KT, ROWS_PER_CORE], fp8, tag="znt0")
        zslab = [
            znt_pool.tile([P, KT, ROWS_PER_CORE], fp8, tag=f"zslab{d}", name=f"zslab{d}")
            for d in range(1, 5)
        ]

        def colview(tile_idx):
            b = tile_idx // 8
            src = znt0 if b == 0 else zslab[b - 1]
            return src, (tile_idx % 8) * P

        acc = accp.tile([P, NTILES * P], bf16, tag="acc")  # colsum accumulators
        accs = accp.tile([P, MB, 9], f32, tag="accs")      # rowsum partials
        posq = accp.tile([P, MB], f32, tag="posq")
        cs = accp.tile([P, 31], f32, tag="cs")             # (a)+(b) exports
        cfold = accp.tile([P, MB], f32, tag="cfold")       # (c) local folds

        for _rep in range(repeat):
            pkt_z_dmas = []
            # ---- phase 0: own rows -> normalized fp8 znT (x16) ----
            for t in range(MB):
                zt = zin.tile([P, 2, D // 2], f32, tag="zt")
                nc.sync.dma_start(
                    out=zt,
                    in_=z_in[t * P : (t + 1) * P, :].rearrange(
                        "p (a b) -> p a b", a=2
                    ),
                )
                stats = small.tile([P, 2, 6], f32, tag="stats")
                nc.vector.bn_stats(out=stats[:, 0, :], in_=zt[:, 0, :])
                nc.vector.bn_stats(out=stats[:, 1, :], in_=zt[:, 1, :])
                mv = small.tile([P, 2], f32, tag="mv")
                nc.vector.bn_aggr(out=mv, in_=stats)
                m2 = small.tile([P, 1], f32, tag="m2")
                nc.vector.tensor_mul(m2, mv[:, 0:1], mv[:, 0:1])
                s2 = small.tile([P, 1], f32, tag="s2")
                nc.vector.tensor_add(s2, m2, mv[:, 1:2])
                nrm = small.tile([P, 1], f32, tag="nrm")
                nc.scalar.activation(nrm, s2, AF.Sqrt, scale=float(D) / (SCALE * SCALE))
                rinv = small.tile([P, 1], f32, tag="rinv")
                nc.vector.reciprocal(rinv, nrm)

                zn_row = work.tile([P, D], bf16, tag="zn_row")
                nc.vector.tensor_scalar_mul(
                    zn_row.rearrange("p (a b) -> p a b", a=2), zt, rinv
                )
                ptr = psum_t.tile([P, KT * P], bf16, tag="ptr")
                for kk in range(KT):
                    nc.tensor.transpose(
                        ptr[:, kk * P : (kk + 1) * P],
                        zn_row[:, kk * P : (kk + 1) * P],
                        ident_bf16,
                    )
                dst = znt0[:, :, t * P : (t + 1) * P]
                src = ptr.rearrange("p (k c) -> p k c", k=KT)
                if t % 2 == 0:
                    nc.scalar.copy(dst, src)
                else:
                    nc.vector.tensor_copy(dst, src)
                pkt_z_dmas.append(
                    nc.sync.dma_start(
                        out=pkt_z[:, :, t * P : (t + 1) * P],
                        in_=znt0[:, :, t * P : (t + 1) * P],
                    )
                )

            # ---- AllGather znT; pull rotated neighbour slabs ----
            cc1 = nc.gpsimd.collective_compute(
                "AllGather",
                mybir.AluOpType.bypass,
                ins=[pkt_z.ap()],
                outs=[gathered_z.ap()],
                replica_groups=[list(range(N_CORES))],
            )
            for pd in pkt_z_dmas:
                _add_dep_helper(cc1.ins, pd.ins, reason="cc1 after pkt_z")
            gz_flat = gathered_z[:, :, :, :].rearrange("s p k c -> (s p) (k c)")
            for d in range(1, 5):
                gd = nc.gpsimd.indirect_dma_start(
                    out=zslab[d - 1][:, :, :].rearrange("p k c -> p (k c)"),
                    out_offset=None,
                    in_=gz_flat,
                    in_offset=bass.IndirectOffsetOnAxis(
                        ap=goff_t[:, d - 1 : d], axis=0
                    ),
                )
                _add_dep_helper(gd.ins, cc1.ins, reason="gather znt after cc1")

            # zero colsum accumulators
            nc.vector.memset(acc, 0.0)
            nc.vector.memset(cfold, 0.0)

            # ---- staircase GEMM + exp rowsums + colsum accumulation ----
            # emit slab-by-slab so slab-0 work runs while the gather lands
            sched = []
            for mb in range(MB):
                for ci, (s, ln) in enumerate(_pieces(mb)):
                    sched.append((s // 1024, mb, ci, s, ln))
            sched.sort(key=lambda x: (x[0], x[1], x[2]))
            for b, mb, ci, s, ln in sched:
                w0 = 128 * mb
                pos_abs = w0 + 32 * P
                lg0 = znt0  # lhsT always from own slab
                if True:
                    src_t = znt0 if b == 0 else zslab[b - 1]
                    r = s % 1024
                    ps = psum_mm.tile([P, 512], f32, tag="ps")
                    for kk in range(0, KT, 2):
                        nc.tensor.matmul(
                            ps[:, :ln],
                            lhsT=lg0[:, kk : kk + 2, w0 : w0 + P],
                            rhs=src_t[:, kk : kk + 2, r : r + ln],
                            perf_mode=mybir.MatmulPerfMode.DoubleRow,
                            start=(kk == 0),
                            stop=(kk == KT - 2),
                        )
                    if s == w0:  # diag tile = first 128 of first chunk
                        nc.vector.copy_predicated(
                            out=ps[:, 0:P], mask=ident_u8, data=negtile
                        )
                    if s <= pos_abs < s + ln:  # pos tile: extract diagonal
                        off = pos_abs - s
                        pos_scr = work.tile([P, P], f32, tag="pos_scr")
                        nc.vector.tensor_mul(
                            pos_scr, ps[:, off : off + P], ident_f32
                        )
                        nc.vector.tensor_reduce(
                            posq[:, mb : mb + 1],
                            pos_scr,
                            axis=mybir.AxisListType.X,
                            op=ALU.add,
                        )
                    ex = work.tile([P, 512], bf16, tag="ex")
                    nc.scalar.activation(
                        ex[:, :ln],
                        ps[:, :ln],
                        AF.Exp,
                        scale=INV_T / (SCALE * SCALE),
                        accum_out=accs[:, mb, ci : ci + 1],
                    )
                    # accumulate into col-tile accumulators, minus diag+pos tiles
                    subs = []
                    a0, a1 = s, s + ln
                    if s == w0:
                        a0 = s + P
                    if s <= pos_abs < s + ln:
                        # pos tile is always the window's last 128 cols
                        if pos_abs == a0:
                            a0 = pos_abs + P
                        else:
                            subs.append((a0, pos_abs))
                            a0 = pos_abs + P
                    if a0 < a1:
                        subs.append((a0, a1))
                    for (u0, u1) in subs:
                        nc.vector.tensor_add(
                            acc[:, u0:u1],
                            acc[:, u0:u1],
                            ex[:, u0 - s : u1 - s],
                        )

            # ---- colsum partition-reduces (ones-matmul) ----
            # (a) v in 8..31 -> cs[:,0:24]; (b) v in 32..38 -> cs[:,24:31]
            for j, v in enumerate(list(range(8, 32)) + list(range(32, 39))):
                cps = psum_cs.tile([P, 1], f32, tag="cps")
                nc.tensor.matmul(
                    cps,
                    lhsT=acc[:, v * P : (v + 1) * P],
                    rhs=ones_col,
                    start=True,
                    stop=True,
                )
                nc.scalar.copy(cs[:, j : j + 1], cps)
            # (c) v in 1..7 -> cfold[:, v]
            for v in range(1, 8):
                cps = psum_cs.tile([P, 1], f32, tag="cps")
                nc.tensor.matmul(
                    cps,
                    lhsT=acc[:, v * P : (v + 1) * P],
                    rhs=ones_col,
                    start=True,
                    stop=True,
                )
                nc.scalar.copy(cfold[:, v : v + 1], cps)

            # ---- pack cs (partition-major) + AllGather ----
            rowsum = accp.tile([P, MB], f32, tag="rowsum")
            nc.vector.tensor_reduce(
                rowsum, accs, axis=mybir.AxisListType.X, op=ALU.add
            )
            nc.vector.tensor_add(rowsum, rowsum, cfold)
            d2 = nc.sync.dma_start(
                out=pkt_dram[:].rearrange("(p s) -> p s", p=P),
                in_=cs,
            )
            cc2 = nc.gpsimd.collective_compute(
                "AllGather",
                mybir.AluOpType.bypass,
                ins=[pkt_dram.ap()],
                outs=[gathered.ap()],
                replica_groups=[list(range(N_CORES))],
            )
            _add_dep_helper(cc2.ins, d2.ins, reason="cc2 after cs")

            # ---- own-rows totals via indirect packet reads; ln; reduce ----
            # gathered flat [(rank p), NSLOT]; partition p pulls rank (c-D)'s
            # line p; columns 8(D-1)..8D-1 are this row-tile-group's slots.
            g2_flat = gathered[:, :].rearrange("s (p q) -> (s p) q", p=P)
            Cd = {
                dd: accp.tile([P, NSLOT], f32, tag=f"Cd{dd}", name=f"Cd{dd}")
                for dd in (1, 2, 3, 4)
            }
            for dd in (1, 2, 3, 4):
                gd2 = nc.gpsimd.indirect_dma_start(
                    out=Cd[dd][:, :],
                    out_offset=None,
                    in_=g2_flat,
                    in_offset=bass.IndirectOffsetOnAxis(
                        ap=poff_t[:, dd - 1 : dd], axis=0
                    ),
                )
                _add_dep_helper(gd2.ins, cc2.ins, reason="read gathered after cc2")
            tot = accp.tile([P, MB], f32, tag="tot")
            nc.vector.tensor_copy(tot, rowsum)
            for dd in (1, 2, 3):
                nc.vector.tensor_add(tot, tot, Cd[dd][:, 8 * (dd - 1) : 8 * dd])
            nc.vector.tensor_add(tot[:, 0:7], tot[:, 0:7], Cd[4][:, 24:31])

            lnt = accp.tile([P, MB], f32, tag="lnt")
            nc.scalar.activation(lnt, tot, AF.Ln)
            gsum = accp.tile([P, 1], f32, tag="gsum")
            nc.vector.tensor_reduce(
                gsum, lnt, axis=mybir.AxisListType.X, op=ALU.add
            )
            poss = accp.tile([P, MB], f32, tag="poss")
            nc.vector.tensor_scalar_mul(poss, posq, -INV_T / (SCALE * SCALE))
            psum_part = accp.tile([P, 1], f32, tag="psum_part")
            nc.vector.tensor_reduce(
                psum_part, poss, axis=mybir.AxisListType.X, op=ALU.add
            )
            part = accp.tile([P, 1], f32, tag="part")
            nc.vector.tensor_add(part, gsum, psum_part)
            nc.sync.dma_start(out=out_dram[:, :], in_=part)

    nc.finalize()
    return nc


def make_in_maps(z):
    z = np.ascontiguousarray(np.asarray(z, dtype=np.float32))
    maps = []
    for c in range(N_CORES):
        goff = np.empty((P, 4), dtype=np.int32)
        poff = np.empty((P, 4), dtype=np.int32)
        for j in range(4):
            goff[:, j] = ((c + 1 + j) % N_CORES) * P + np.arange(P)
            poff[:, j] = ((c - 1 - j) % N_CORES) * P + np.arange(P)
        maps.append(
            {
                "z": np.ascontiguousarray(
                    z[c * ROWS_PER_CORE : (c + 1) * ROWS_PER_CORE]
                ),
                "goff": goff,
                "poff": poff,
            }
        )
    return maps


def _get_nc():
    if "nc" not in _CACHE:
        _CACHE["nc"] = _build_nc()
    return _CACHE["nc"]


def _run(z, trace=False):
    from concourse.bass_utils import run_bass_kernel_spmd

    z = np.ascontiguousarray(np.asarray(z, dtype=np.float32))
    assert z.shape == (N, D), z.shape
    nc = _get_nc()
    res = run_bass_kernel_spmd(
        nc, make_in_maps(z), core_ids=list(range(N_CORES)), trace=False
    )
    total = np.float64(0.0)
    for r in res.results:
        total += r["out"].astype(np.float64).sum()
    loss = np.float32(total / N)
    return loss, res


def kernel(z):
    loss, _ = _run(z, trace=False)
    return np.array(loss, dtype=np.float32)
